# revision 11
# baseline (speedup 1.0000x reference)
"""Trainium2 Bass kernel for nn_Conv2dLocal_47132971106931.

The reference unfolds (1,128,256,256) -> (1, C*9, L), permutes and
*raw-reshapes* to (1, C, L, 9), multiplies by per-location weights (L, 9)
and sums the tap axis.  The raw reshape scrambles indices; the true math is

  out[0,c,y,x] = sum_k xpad[x%128, 2c+s+i_k, ((2y+t)%256)+j_k] * w[y*256+x, k]
      s = [y>=128], t = [x>=128], (i_k,j_k) = divmod(k,3)

In coordinates u = c (0..127), rho = s, q' = 2*(y%128)+t (0..255), a = x%128:

  O[rho,q',a,u] = sum_k xpad[a, 2u+rho+i_k, q'+j_k] * W[rho,q',a,k]
  W[rho,q',a,:] = w[rho*32768 + (q'//2)*256 + (q'%2)*128 + a, :]

Sharding: q' is split across the 8 cores (32 columns each, + halo).

Compute runs on the 128x128 TensorEngine: for each tile tau=(rho,q') the
9-tap weighted sum over k becomes PSUM-accumulating matmuls whose
stationary operands are block-delta weight matrices:

  wave A (taps 0-3):  lhsT[(kk*32+ar), a] = [ar==a%32] * W[.,.,a,kk]
  wave B (taps 4-7):  same with k=4+kk
  wave C (tap 8):     lhsT[ar, a%32]     = [ar==a%32] * W[.,.,a,8]  (K=32)

and the moving operands are host-side pre-unfolded bf16 copies of x
(partition p=(kk*32+ar) holds plane a'=32b+ar shifted by (i_k,j_k), with
free layout (q', rho, u) so every matmul streams 128 stride-1 columns).
Each (tau, a-block b) matmul writes psum partitions [32b,32b+32) -- the
four blocks run concurrently on distinct column-groups of the PE array.
ScalarE drains 4 tiles per instruction (psum fp32 -> SBUF bf16, strided
across 4 psum banks); one DMA stores the result on the last rep.
"""

import sys
import numpy as np

TRN_REPO = "/opt/trn_rl_repo"

# ---- problem geometry (hardcoded) ----
C = 128
H = W = 256
NCORES = 8
QS = 32          # q' columns per core
NTAU = 64        # tiles per core: (rho in 2) x (q' in 32)
NU = 128         # u = output-channel dim

_CACHE = {}


def _build_nc(reps=1):
    sys.path.insert(0, TRN_REPO)
    import concourse.bass as bass
    import concourse.bacc as bacc
    import concourse.mybir as mybir
    from concourse.tile import TileContext

    f32 = mybir.dt.float32
    bf16 = mybir.dt.bfloat16
    nc = bacc.Bacc("TRN2", target_bir_lowering=False, debug=False)

    # all inputs packed into one tensor -> one DMA -> every matmul needs
    # at most one sync wait (walrus rejects multi-wait matmuls)
    # layout: xu[8*8192] | xn[8192] | sa[8192] | sb[8192] | sc[2048]
    NIN = 8 * 8192 + 8192 + 8192 + 8192 + 2048
    allin = nc.dram_tensor("allin", (C, NIN), bf16, kind="ExternalInput").ap()
    y_out = nc.dram_tensor("y", (C, NTAU * 128), bf16, kind="ExternalOutput").ap()

    with TileContext(nc) as tc:
        with (
            tc.tile_pool(name="inp", bufs=1) as inpool,
            tc.tile_pool(name="yp", bufs=1) as ypool,
            tc.tile_pool(name="psp", bufs=2, space="PSUM") as pspool,
        ):
            ain = inpool.tile([C, NIN], bf16)
            ysb = ypool.tile([C, NTAU * 128], bf16)

            XU0, XN0, SA0, SB0, SC0 = 0, 65536, 73728, 81920, 90112

            nc.sync.dma_start(out=ain[:, :], in_=allin[:, :])

            y3 = ysb.rearrange("p (g t u) -> p g t u", g=16, t=4, u=128)

            for rep in range(reps):
                for grp in range(16):
                    ps = pspool.tile([C, 4 * 512], f32, tag="ps")
                    ps3 = ps.rearrange("p (t z) -> p t z", t=4, z=512)
                    for tloc in range(4):
                        tau = grp * 4 + tloc
                        q, s = divmod(tau, 2)
                        # tau ordering: (q', rho) fastest on rho
                        moff = q * 256 + s * 128
                        for b in range(4):
                            o = SA0 + tau * 128 + 32 * b
                            nc.tensor.matmul(
                                ps3[32 * b:32 * b + 32, tloc, 0:128],
                                ain[:, o:o + 32],
                                ain[:, XU0 + b * 8192 + moff:
                                    XU0 + b * 8192 + moff + 128],
                                start=True, stop=False,
                                tile_position=(0, 32 * b),
                            )
                        for b in range(4):
                            o = SB0 + tau * 128 + 32 * b
                            nc.tensor.matmul(
                                ps3[32 * b:32 * b + 32, tloc, 0:128],
                                ain[:, o:o + 32],
                                ain[:, XU0 + (4 + b) * 8192 + moff:
                                    XU0 + (4 + b) * 8192 + moff + 128],
                                start=False, stop=False,
                                tile_position=(0, 32 * b),
                            )
                        for b in range(4):
                            o = SC0 + tau * 32
                            nc.tensor.matmul(
                                ps3[32 * b:32 * b + 32, tloc, 0:128],
                                ain[32 * b:32 * b + 32, o:o + 32],
                                ain[32 * b:32 * b + 32, XN0 + moff:XN0 + moff + 128],
                                start=False, stop=True,
                                tile_position=(32 * b, 32 * b),
                            )
                    nc.scalar.copy(out=y3[:, grp, :, :], in_=ps3[:, :, 0:128])
                if rep == reps - 1:
                    nc.sync.dma_start(out=y_out[:, :], in_=ysb[:, :])
    nc.finalize()
    return nc


def _get_nc(reps=1):
    key = ("nc", reps)
    if key not in _CACHE:
        _CACHE[key] = _build_nc(reps=reps)
    return _CACHE[key]


def _prep_inputs(input_tensor, weights):
    import ml_dtypes
    bf16 = ml_dtypes.bfloat16

    x = np.ascontiguousarray(np.asarray(input_tensor, dtype=np.float32))
    w = np.ascontiguousarray(np.asarray(weights, dtype=np.float32))
    xpad = np.pad(x[0], ((0, 0), (1, 1), (1, 1)))  # (128, 258, 258)

    in_maps = []
    ar = np.arange(32)
    for m in range(NCORES):
        q0 = QS * m
        # --- moving operands ---
        xu = np.empty((C, 8 * 8192), bf16)
        for g in range(2):
            for b in range(4):
                parts = []
                for kk in range(4):
                    k = 4 * g + kk
                    i, j = divmod(k, 3)
                    sub = xpad[32 * b:32 * b + 32, i:i + 256, q0 + j:q0 + j + QS]
                    sub = sub.reshape(32, 128, 2, QS).transpose(0, 3, 2, 1)
                    parts.append(sub.reshape(32, 8192))
                xu[:, (g * 4 + b) * 8192:(g * 4 + b + 1) * 8192] = \
                    np.concatenate(parts, axis=0).astype(bf16)
        sub = xpad[:, 2:258, q0 + 2:q0 + 2 + QS]
        xn = np.ascontiguousarray(
            sub.reshape(C, 128, 2, QS).transpose(0, 3, 2, 1).reshape(C, 8192)
        ).astype(bf16)

        # --- weights W[s, q, a, k] ---
        s_ = np.arange(2)[:, None, None]
        q_ = np.arange(QS)[None, :, None]
        a_ = np.arange(C)[None, None, :]
        l_idx = s_ * 32768 + ((q0 + q_) // 2) * 256 + ((q0 + q_) % 2) * 128 + a_
        Wm = w[l_idx]  # (2, QS, 128, 9)

        sa = np.zeros((C, NTAU * 128), np.float32)
        sb = np.zeros((C, NTAU * 128), np.float32)
        sc = np.zeros((C, NTAU * 32), np.float32)
        for s in range(2):
            for q in range(QS):
                tau = q * 2 + s
                for kk in range(4):
                    for b in range(4):
                        sa[kk * 32 + ar, tau * 128 + b * 32 + ar] = Wm[s, q, 32 * b + ar, kk]
                        sb[kk * 32 + ar, tau * 128 + b * 32 + ar] = Wm[s, q, 32 * b + ar, 4 + kk]
                for b in range(4):
                    sc[32 * b + ar, tau * 32 + ar] = Wm[s, q, 32 * b + ar, 8]

        allin = np.concatenate(
            [xu, xn, sa.astype(bf16), sb.astype(bf16), sc.astype(bf16)],
            axis=1,
        )
        in_maps.append({"allin": np.ascontiguousarray(allin)})
    return in_maps


def _gather_output(results):
    out = np.empty((C, H, W), np.float32)
    for m in range(NCORES):
        y = np.asarray(results[m]["y"], dtype=np.float32)
        # y[a, tau*128+u], tau = q*2+s -> [a, q, s, u] ; q = 2v+t
        arr = y.reshape(C, 16, 2, 2, 128)          # [a, v, t, s, u]
        arr = arr.transpose(3, 1, 2, 4, 0)          # [s, v, t, u, a]
        for s in range(2):
            for t in range(2):
                out[:, 128 * s + 16 * m: 128 * s + 16 * m + 16,
                    128 * t: 128 * t + 128] = arr[s, :, t].transpose(1, 0, 2)
    return out.reshape(1, C, H, W)


def _run(in_maps, trace=False):
    sys.path.insert(0, TRN_REPO)
    from concourse.bass_utils import run_bass_kernel_spmd

    nc = _get_nc()
    res = run_bass_kernel_spmd(
        nc, in_maps, core_ids=list(range(NCORES)), trace=trace
    )
    return res


def kernel(input_tensor, weights):
    in_maps = _prep_inputs(input_tensor, weights)
    res = _run(in_maps, trace=False)
    return _gather_output(res.results)


def bench(input_tensor, weights, trace=True):
    in_maps = _prep_inputs(input_tensor, weights)
    res = _run(in_maps, trace=trace)
    return _gather_output(res.results), res


def _make_runner(nc, in_maps):
    """Build a reusable jitted 8-core runner for a prebuilt nc."""
    sys.path.insert(0, TRN_REPO)
    import jax
    import numpy as np_
    from jax.sharding import Mesh, PartitionSpec
    from jax.experimental.shard_map import shard_map
    from concourse import bass2jax
    import concourse.mybir as mybir

    bass2jax.install_neuronx_cc_hook()

    partition_name = (
        nc.partition_id_tensor.name if nc.partition_id_tensor else None
    )
    in_names, out_names, out_avals, zero_outs = [], [], [], []
    for alloc in nc.m.functions[0].allocations:
        if not isinstance(alloc, mybir.MemoryLocationSet):
            continue
        name = alloc.memorylocations[0].name
        if alloc.kind == "ExternalInput":
            if name != partition_name:
                in_names.append(name)
        elif alloc.kind == "ExternalOutput":
            shape = tuple(alloc.tensor_shape)
            dtype = mybir.dt.np(alloc.dtype)
            out_avals.append(jax.core.ShapedArray(shape, dtype))
            out_names.append(name)
            zero_outs.append(np_.zeros(shape, dtype))
    n_params = len(in_names)
    n_outs = len(out_names)
    all_in_names = list(in_names) + list(out_names)
    if partition_name is not None:
        all_in_names.append(partition_name)

    def _body(*args):
        ins = list(args[:n_params])
        outs = list(args[n_params:])
        pid = [bass2jax.partition_id_tensor()] if partition_name else []
        outs = list(bass2jax._bass_exec_p.bind(
            *ins, *outs, *pid,
            out_avals=tuple(out_avals),
            in_names=tuple(all_in_names),
            out_names=tuple(out_names),
            lowering_input_output_aliases=(),
            sim_require_finite=True,
            sim_require_nnan=True,
            nc=nc,
        ))
        return tuple(outs)

    devices = jax.devices()[:NCORES]
    mesh = Mesh(np_.asarray(devices), ("core",))
    in_specs = (PartitionSpec("core"),) * (n_params + n_outs)
    out_specs = (PartitionSpec("core"),) * n_outs
    donate = tuple(range(n_params, n_params + n_outs))

    per_core = [[np_.asarray(m[nm]) for nm in in_names] for m in in_maps]
    concat_in = [
        np_.concatenate([per_core[c][i] for c in range(NCORES)], axis=0)
        for i in range(n_params)
    ]
    concat_zeros = [
        np_.zeros((NCORES * z.shape[0], *z.shape[1:]), z.dtype)
        for z in zero_outs
    ]

    f = jax.jit(
        shard_map(_body, mesh=mesh, in_specs=in_specs,
                  out_specs=out_specs, check_rep=False),
        donate_argnums=donate, keep_unused=True,
    )
    cin = [jax.device_put(a) for a in concat_in]
    state = {"outs": None}

    def call():
        prev = state["outs"]
        if prev is None:
            prev = [jax.device_put(z) for z in concat_zeros]
        outs = f(*cin, *prev)
        for o in outs:
            o.block_until_ready()
        state["outs"] = list(outs)
        return outs

    def gather(outs):
        return [
            {nm: np_.asarray(outs[i]).reshape(NCORES, *out_avals[i].shape)[c]
             for i, nm in enumerate(out_names)}
            for c in range(NCORES)
        ]

    return call, gather


def time_kernel(input_tensor, weights, k_long=11, reps=8):
    """Per-iteration device time via in-NEFF repetition: build the same
    program with the compute+store body repeated K times (inputs loaded
    once), then dt = (t_K - t_1) / (K - 1) cancels the proxy round-trip
    and NEFF launch overhead.

    Returns (dt_seconds, t1_seconds, out_full_from_k_run)."""
    import time as _time
    in_maps = _prep_inputs(input_tensor, weights)
    call1, gather1 = _make_runner(_get_nc(reps=1), in_maps)
    callk, gatherk = _make_runner(_get_nc(reps=k_long), in_maps)

    call1(); callk()  # compile + warm
    t1s, tks = [], []
    outs_k = None
    for _ in range(reps):
        t0 = _time.perf_counter()
        call1()
        t1s.append(_time.perf_counter() - t0)
        t0 = _time.perf_counter()
        outs_k = callk()
        tks.append(_time.perf_counter() - t0)
    dt = (min(tks) - min(t1s)) / (k_long - 1)
    print(f"[time_kernel] t1 samples (ms): {[round(t*1e3,2) for t in t1s]}")
    print(f"[time_kernel] t{k_long} samples (ms): {[round(t*1e3,2) for t in tks]}")
    return dt, min(t1s), _gather_output(gatherk(outs_k))
